# revision 8
# baseline (speedup 1.0000x reference)
"""TRN2 Bass kernel for nn_MultiBlockStructuredScoreNet.

Computes s(z) = -grad_z U(z) where
  U(z) = sum_k MLP_k(z_k) + sum_r z_8^T W_r z_{8-r}
for z of shape (8192, 9*256), data-parallel over 8 NeuronCores.

Strategy (per core, 1024 samples):
 - Host pre-transposes z to neuron-major (zT) so the PE can contract over
   the neuron dim without on-chip transposes; host also packs / transposes
   / sign-flips the small parameters so the kernel's PSUM accumulation
   directly yields the score (no negate pass).
 - All matmuls run as float32r (full fp32 storage, single-pass PE rate,
   ~1.5e-4 operand rounding) or bf16 (MM_MODE switch).
 - MLP: u1 via col-tiled [K=128,M=32] matmuls packing 4 blocks per PSUM
   tile; u2 / dh1 via block-diagonal [128,128] weights; SiLU and SiLU'
   straight from the ACT table (Silu / Derivative_silu) with fused bias.
 - Cross terms + MLP d_z accumulate per 128-sample chunk into PSUM tiles
   holding the final b-major output, copied to SBUF and DMA'd out.
"""

import numpy as np
import ml_dtypes

import concourse.bass as bass
import concourse.tile as tile
from concourse import bacc, mybir
from concourse.bass_utils import run_bass_kernel_spmd

AF = mybir.ActivationFunctionType
F32 = mybir.dt.float32

N_CORES = 8
BATCH = 8192
B_CORE = BATCH // N_CORES     # 1024
BT = 512                      # batch tile (PSUM free-dim max for f32)
NBT = B_CORE // BT            # 2 batch tiles per core
NCHUNK = 4                    # 128-sample chunks per batch tile
NB = 9                        # blocks
P_MAX = 8
NN = 256                      # neurons per block
NH = NN // 128                # 2 partition-halves per block
D = NB * NN                   # 2304
H = 32

MM_MODE = "f32r"              # "f32r" | "bf16"


def _mm_np_dtype():
    return np.float32 if MM_MODE == "f32r" else ml_dtypes.bfloat16


def _mm_store_dt():
    return mybir.dt.float32r if MM_MODE == "f32r" else mybir.dt.bfloat16


def _mm(ap):
    return ap


# ---------------------------------------------------------------- kernel body

def _body(tc, out, zt, w1, w1t, w2bd, w2tbd, biases, wlag, wfut, ctx):
    nc = tc.nc
    sdt = _mm_store_dt()

    const = ctx.enter_context(tc.tile_pool(name="const", bufs=1))
    ztp = ctx.enter_context(tc.tile_pool(name="zt", bufs=2 * 2 * NB))
    mlpp = ctx.enter_context(tc.tile_pool(name="mlpp", bufs=3, space="PSUM"))
    actp = ctx.enter_context(tc.tile_pool(name="actp", bufs=8))
    du1p = ctx.enter_context(tc.tile_pool(name="du1p", bufs=6))
    outp = ctx.enter_context(tc.tile_pool(name="outp", bufs=5, space="PSUM"))
    outs = ctx.enter_context(tc.tile_pool(name="outs", bufs=3))

    # ---- parameters -> SBUF (resident for the whole kernel)
    # stacked u1 weights: per (group, block-in-group, n-half) a [128, P_g]
    # lhsT whose cols 32j..32j+32 hold W1_k[half]; zeros elsewhere.  This
    # keeps every u1 matmul full-width at PSUM base 0 (f32r requirement).
    w1_sb = [const.tile([128, 128], sdt, tag=f"w1s{i}", name=f"w1s{i}")
             for i in range(2 * NB)]
    for i in range(2 * NB):
        nc.sync.dma_start(w1_sb[i][:], w1[i])
    bias_sb = const.tile([128, 9], F32, name="biassb")
    nc.sync.dma_start(bias_sb[:], biases[:])
    w1t_sb = [const.tile([128, NN], sdt, tag=f"w1t{g}", name=f"w1t{g}") for g in range(3)]
    w2bd_sb = [const.tile([128, 128], sdt, tag=f"w2bd{g}", name=f"w2bd{g}") for g in range(3)]
    w2tbd_sb = [const.tile([128, 128], sdt, tag=f"w2tbd{g}", name=f"w2tbd{g}") for g in range(3)]
    for g in range(3):
        nc.sync.dma_start(w1t_sb[g][:], w1t[g])
        nc.sync.dma_start(w2bd_sb[g][:], w2bd[g])
        nc.sync.dma_start(w2tbd_sb[g][:], w2tbd[g])
    wlag_sb = [const.tile([128, NN], sdt, tag=f"wl{i}", name=f"wl{i}") for i in range(2 * P_MAX)]
    wfut_sb = [const.tile([128, NN], sdt, tag=f"wf{i}", name=f"wf{i}") for i in range(2 * P_MAX)]
    for i in range(2 * P_MAX):
        nc.sync.dma_start(wlag_sb[i][:], wlag[i])
        nc.sync.dma_start(wfut_sb[i][:], wfut[i])

    for t in range(NBT):
        # ---- zT tiles for this batch tile: one [128, BT] per 128-neuron chunk
        zt_sb = [ztp.tile([128, BT], sdt, tag="zt", name="ztsb") for _ in range(2 * NB)]
        for c in range(2 * NB):
            nc.sync.dma_start(zt_sb[c][:], zt[t, c])

        # ---- per-block MLP forward + backward (through du1)
        du1_sb = []
        for g in range(3):
            nblk = 4 if g < 2 else 1
            P = 32 * nblk
            u1 = mlpp.tile([128, BT], F32, tag="mlpp", name="mlppt")
            nmm = 2 * nblk
            for j in range(nblk):
                k = 4 * g + j
                for hf in range(2):
                    i = 2 * j + hf
                    nc.tensor.matmul(
                        u1[:P, :],
                        _mm(w1_sb[2 * k + hf][:, :P]),
                        _mm(zt_sb[2 * k + hf][:]),
                        start=(i == 0), stop=(i == nmm - 1),
                    )
            h1 = actp.tile([128, BT], sdt, tag="act", name="actt")
            nc.scalar.activation(h1[:P], u1[:P], AF.Silu, bias=bias_sb[:P, g:g + 1])
            sp1 = actp.tile([128, BT], F32, tag="act", name="actf")
            nc.scalar.activation(
                sp1[:P], u1[:P], AF.Derivative_silu, bias=bias_sb[:P, g:g + 1])
            u2 = mlpp.tile([128, BT], F32, tag="mlpp", name="mlppt")
            nc.tensor.matmul(
                u2[:P], _mm(w2bd_sb[g][:P, :P]), _mm(h1[:P]), start=True, stop=True)
            sp2 = actp.tile([128, BT], F32, tag="act", name="actf")
            nc.scalar.activation(
                sp2[:P], u2[:P], AF.Derivative_silu, bias=bias_sb[:P, 3 + g:4 + g])
            du2 = actp.tile([128, BT], sdt, tag="act", name="actt")
            # x (-gW3): from here on everything carries the output's minus sign
            nc.vector.tensor_scalar_mul(du2[:P], sp2[:P], bias_sb[:P, 6 + g:7 + g])
            dh1 = mlpp.tile([128, BT], F32, tag="mlpp", name="mlppt")
            nc.tensor.matmul(
                dh1[:P], _mm(w2tbd_sb[g][:P, :P]), _mm(du2[:P]), start=True, stop=True)
            du1 = du1p.tile([128, BT], sdt, tag="du1", name="du1t")
            nc.vector.tensor_mul(du1[:P], dh1[:P], sp1[:P])
            du1_sb.append(du1)

        # ---- cross couplings + MLP d_z, accumulated b-major per 128-chunk
        for c in range(NCHUNK):
            bs = slice(128 * c, 128 * c + 128)
            op = [outp.tile([128, 512], F32, tag="outp", name="outpt") for _ in range(4)]
            o8 = outp.tile([128, 512], F32, tag="outp", name="outpt")
            # blocks 0..7: dz_k = z_fut @ W_r (r = 8-k), + MLP term
            for k in range(P_MAX):
                pt = op[k // 2][:, 256 * (k % 2):256 * (k % 2) + 256]
                for ih in range(2):
                    nc.tensor.matmul(
                        pt, _mm(zt_sb[2 * P_MAX + ih][:, bs]),
                        _mm(wlag_sb[2 * (7 - k) + ih][:]),
                        start=(ih == 0), stop=False)
                g, j = k // 4, k % 4
                nc.tensor.matmul(
                    pt, _mm(du1_sb[g][32 * j:32 * j + 32, bs]),
                    _mm(w1t_sb[g][32 * j:32 * j + 32, :]),
                    start=False, stop=True, tile_position=(32 * j, 0))
            # block 8: dz_8 = sum_r lags_r @ W_r^T, + MLP term
            for r in range(1, P_MAX + 1):
                for jh in range(2):
                    nc.tensor.matmul(
                        o8[:, :NN], _mm(zt_sb[2 * (P_MAX - r) + jh][:, bs]),
                        _mm(wfut_sb[2 * (r - 1) + jh][:]),
                        start=(r == 1 and jh == 0), stop=False)
            nc.tensor.matmul(
                o8[:, :NN], _mm(du1_sb[2][0:32, bs]), _mm(w1t_sb[2][0:32, :]),
                start=False, stop=True, tile_position=(0, 0))

            # ---- PSUM -> SBUF, assemble the [128, 2304] output rows, DMA out
            ot = outs.tile([128, D], F32, tag="outs", name="outst")
            for m in range(4):
                nc.any.tensor_copy(ot[:, 512 * m:512 * (m + 1)], op[m][:])
            nc.any.tensor_copy(ot[:, 2048:2304], o8[:, :NN])
            r0 = t * BT + c * 128
            nc.sync.dma_start(out[r0:r0 + 128, :], ot[:])


# ------------------------------------------------------------- build + launch

_CACHED = {}


def _build():
    key = MM_MODE
    if key in _CACHED:
        return _CACHED[key]
    sdt = _mm_store_dt()
    nc = bacc.Bacc("TRN2", target_bir_lowering=False, debug=False,
                   num_devices=N_CORES)
    zt = nc.dram_tensor("zt", [NBT, 2 * NB, 128, BT], sdt, kind="ExternalInput").ap()
    w1 = nc.dram_tensor("w1", [2 * NB, 128, 128], sdt, kind="ExternalInput").ap()
    w1t = nc.dram_tensor("w1t", [3, 128, NN], sdt, kind="ExternalInput").ap()
    w2bd = nc.dram_tensor("w2bd", [3, 128, 128], sdt, kind="ExternalInput").ap()
    w2tbd = nc.dram_tensor("w2tbd", [3, 128, 128], sdt, kind="ExternalInput").ap()
    biases = nc.dram_tensor("biases", [128, 9], F32, kind="ExternalInput").ap()
    wlag = nc.dram_tensor("wlag", [2 * P_MAX, 128, NN], sdt, kind="ExternalInput").ap()
    wfut = nc.dram_tensor("wfut", [2 * P_MAX, 128, NN], sdt, kind="ExternalInput").ap()
    out = nc.dram_tensor("out", [B_CORE, D], F32, kind="ExternalOutput").ap()

    from contextlib import ExitStack
    with tile.TileContext(nc) as tc:
        with ExitStack() as ctx:
            _body(tc, out, zt, w1, w1t, w2bd, w2tbd, biases, wlag, wfut, ctx)
    nc.compile()
    _CACHED[key] = nc
    return nc


def _prep_params(gW1, gb1, gW2, gb2, gW3, gb3, W):
    mdt = _mm_np_dtype()
    # w1[2k+hf, p, 32j+h] = gW1[k, 128*hf+p, h]  (j = k%4; other cols zero)
    w1 = np.zeros((2 * NB, 128, 128), np.float32)
    for k in range(NB):
        j = k % 4
        for hf in range(2):
            w1[2 * k + hf, :, 32 * j:32 * j + 32] = gW1[k, 128 * hf:128 * (hf + 1), :]
    w1 = w1.astype(mdt)
    w1t = np.zeros((3, 128, NN), np.float32)
    w2bd = np.zeros((3, 128, 128), np.float32)
    w2tbd = np.zeros((3, 128, 128), np.float32)
    biases = np.zeros((128, 9), np.float32)
    for k in range(NB):
        g, j = k // 4, k % 4
        rs = slice(32 * j, 32 * j + 32)
        w1t[g, rs, :] = gW1[k].T                    # [h, n]
        w2bd[g, rs, rs] = gW2[k]                    # lhsT for u2 = W2^T.T @ h1
        w2tbd[g, rs, rs] = gW2[k].T                 # lhsT for dh1
        biases[rs, g] = gb1[k]
        biases[rs, 3 + g] = gb2[k]
        biases[rs, 6 + g] = -gW3[k]                 # minus sign enters here
    wlag = np.zeros((2 * P_MAX, 128, NN), np.float32)
    wfut = np.zeros((2 * P_MAX, 128, NN), np.float32)
    for r in range(1, P_MAX + 1):
        for hf in range(2):
            wlag[2 * (r - 1) + hf] = -W[r - 1][128 * hf:128 * (hf + 1), :]
            wfut[2 * (r - 1) + hf] = -W[r - 1].T[128 * hf:128 * (hf + 1), :]
    return {
        "w1": w1, "w1t": w1t.astype(mdt), "w2bd": w2bd.astype(mdt),
        "w2tbd": w2tbd.astype(mdt), "biases": biases,
        "wlag": wlag.astype(mdt), "wfut": wfut.astype(mdt),
    }


def run(inputs, trace=False):
    nc = _build()
    mdt = _mm_np_dtype()
    params = _prep_params(
        np.asarray(inputs["gW1"]), np.asarray(inputs["gb1"]),
        np.asarray(inputs["gW2"]), np.asarray(inputs["gb2"]),
        np.asarray(inputs["gW3"]), np.asarray(inputs["gb3"]),
        np.asarray(inputs["W"]))
    z = np.asarray(inputs["z"])
    in_maps = []
    for ci in range(N_CORES):
        zc = z[ci * B_CORE:(ci + 1) * B_CORE]
        # zt[t, c, p, b] = zc[BT*t + b, 128*c + p]
        ztc = np.ascontiguousarray(
            zc.reshape(NBT, BT, 2 * NB, 128).transpose(0, 2, 3, 1)).astype(mdt)
        in_maps.append({"zt": ztc, **params})
    res = run_bass_kernel_spmd(nc, in_maps, core_ids=list(range(N_CORES)),
                               trace=trace)
    out = np.concatenate([r["out"] for r in res.results], axis=0)
    return out, res


def kernel(**inputs) -> np.ndarray:
    out, _ = run(inputs, trace=False)
    return out


# revision 9
# speedup vs baseline: 1.6671x; 1.6671x over previous
"""TRN2 Bass kernel for nn_MultiBlockStructuredScoreNet.

Computes s(z) = -grad_z U(z) where
  U(z) = sum_k MLP_k(z_k) + sum_r z_8^T W_r z_{8-r}
for z of shape (8192, 9*256), data-parallel over 8 NeuronCores.

Per core (1024 samples):
 - Host pre-transposes z to neuron-major (zT) so the PE contracts over the
   neuron dim with no on-chip transposes; host packs/transposes/sign-flips
   the small parameters so PSUM accumulation directly yields the score.
 - MM_MODE picks the PE dtype: fp16 (default: full PE rate, ~2.5e-4),
   bf16 (~2e-3), or f32r (fp32 storage, ~1.3e-4 but slower weight loads).
 - Inputs ship as 4 packed DRAM tensors -> ~10 big DMAs (per-DMA overhead
   on the HWDGE queues is ~0.6us, so DMA count matters more than bytes).
 - MLP: u1 via col-tiled [K=128,M=32] matmuls packing 4 blocks per PSUM
   tile (f32r falls back to stacked zero-padded weights: its matmuls must
   write PSUM partition 0); u2/dh1 via block-diagonal [128,128] weights;
   SiLU / SiLU' from the ACT table with fused bias, batched by function
   so the ACT engine doesn't thrash table reloads.
 - Cross couplings + MLP d_z accumulate b-major into PSUM per 128-sample
   chunk; PSUM->SBUF copies are split across DVE/ACT; one 1.2MB DMA per
   chunk stores the final rows.
"""

import numpy as np
import ml_dtypes

import concourse.bass as bass
import concourse.tile as tile
from concourse import bacc, mybir
from concourse.bass_utils import run_bass_kernel_spmd

AF = mybir.ActivationFunctionType
F32 = mybir.dt.float32

N_CORES = 8
BATCH = 8192
B_CORE = BATCH // N_CORES     # 1024
BT = 512                      # batch tile (PSUM free-dim max for f32)
NBT = B_CORE // BT            # 2 batch tiles per core
NCHUNK = 4                    # 128-sample chunks per batch tile
NB = 9                        # blocks
P_MAX = 8
NN = 256                      # neurons per block
D = NB * NN                   # 2304
H = 32

ZW = 2 * NB * BT              # zt tile cols per batch tile: 18 chunks x 512
PW = 2 * NB * 128 + 3 * NN + 3 * 128 + 3 * 128   # params cols: 3840
CW = 2 * P_MAX * NN * 2       # cross params cols: wlag + wfut = 8192
OFF_W1T = 2 * NB * 128        # 2304
OFF_W2BD = OFF_W1T + 3 * NN   # 3072
OFF_W2TBD = OFF_W2BD + 3 * 128  # 3456
OFF_WFUT = 2 * P_MAX * NN     # 4096

MM_MODE = "fp16"              # "fp16" | "bf16" | "f32r"

_DT = {
    "fp16": (mybir.dt.float16, np.float16),
    "bf16": (mybir.dt.bfloat16, ml_dtypes.bfloat16),
    "f32r": (mybir.dt.float32r, np.float32),
}


def _body(tc, out, zt, params, cparams, biases, ctx):
    nc = tc.nc
    sdt = _DT[MM_MODE][0]

    const = ctx.enter_context(tc.tile_pool(name="const", bufs=1))
    ztp = ctx.enter_context(tc.tile_pool(name="ztp", bufs=2))
    mlpp = ctx.enter_context(tc.tile_pool(name="mlpp", bufs=3, space="PSUM"))
    actp = ctx.enter_context(tc.tile_pool(name="actp", bufs=8))
    du1p = ctx.enter_context(tc.tile_pool(name="du1p", bufs=6))
    outp = ctx.enter_context(tc.tile_pool(name="outp", bufs=5, space="PSUM"))
    outs = ctx.enter_context(tc.tile_pool(name="outs", bufs=3))

    # ---- inputs -> SBUF (few big DMAs; ordered so compute starts early)
    pa_sb = const.tile([128, PW], sdt, name="pa")
    nc.sync.dma_start(pa_sb[:], params[:])
    bias_sb = const.tile([128, 9], F32, name="biassb")
    nc.sync.dma_start(bias_sb[:], biases[:])
    zt_sb = [ztp.tile([128, ZW], sdt, tag="zt", name="ztsb") for _ in range(NBT)]
    SPLITS = [(0, 8 * BT), (8 * BT, 16 * BT), (16 * BT, ZW)]
    for a, b in SPLITS:
        nc.sync.dma_start(zt_sb[0][:, a:b], zt[0, :, a:b])
    cp_sb = const.tile([128, CW], sdt, name="cp")
    nc.sync.dma_start(cp_sb[:], cparams[:])
    for a, b in SPLITS:
        nc.sync.dma_start(zt_sb[1][:, a:b], zt[1, :, a:b])

    def ztc(t, c):            # zT chunk c (128 neurons) of batch tile t
        return zt_sb[t][:, BT * c:BT * (c + 1)]

    for t in range(NBT):
        # ---- per-block MLPs, batched by ACT function across the 3 groups
        u1s, h1s, sp1s = [], [], []
        for g in range(3):
            nblk = 4 if g < 2 else 1
            P = 32 * nblk
            u1 = mlpp.tile([128, BT], F32, tag="mlpp", name="u1t")
            for j in range(nblk):
                k = 4 * g + j
                for hf in range(2):
                    c = 2 * k + hf
                    if MM_MODE == "f32r":
                        nc.tensor.matmul(
                            u1[:P, :], pa_sb[:, 128 * c:128 * c + P], ztc(t, c),
                            start=(2 * j + hf == 0), stop=(2 * j + hf == 2 * nblk - 1))
                    else:
                        nc.tensor.matmul(
                            u1[32 * j:32 * j + 32, :],
                            pa_sb[:, 128 * c + 32 * j:128 * c + 32 * j + 32],
                            ztc(t, c), start=(hf == 0), stop=(hf == 1),
                            tile_position=(0, 32 * j))
            u1s.append((u1, P))
        for g in range(3):
            u1, P = u1s[g]
            h1 = actp.tile([128, BT], sdt, tag="act", name="h1t")
            nc.scalar.activation(h1[:P], u1[:P], AF.Silu, bias=bias_sb[:P, g:g + 1])
            h1s.append(h1)
        for g in range(3):
            u1, P = u1s[g]
            sp1 = actp.tile([128, BT], F32, tag="act", name="sp1t")
            nc.scalar.activation(sp1[:P], u1[:P], AF.Derivative_silu,
                                 bias=bias_sb[:P, g:g + 1])
            sp1s.append(sp1)
        du1_sb = []
        for g in range(3):
            P = u1s[g][1]
            u2 = mlpp.tile([128, BT], F32, tag="mlpp", name="u2t")
            nc.tensor.matmul(u2[:P], pa_sb[:P, OFF_W2BD + 128 * g:OFF_W2BD + 128 * g + P],
                             h1s[g][:P], start=True, stop=True)
            sp2 = actp.tile([128, BT], F32, tag="act", name="sp2t")
            nc.scalar.activation(sp2[:P], u2[:P], AF.Derivative_silu,
                                 bias=bias_sb[:P, 3 + g:4 + g])
            du2 = actp.tile([128, BT], sdt, tag="act", name="du2t")
            # x (-gW3): from here the output's minus sign is carried along
            nc.vector.tensor_scalar_mul(du2[:P], sp2[:P], bias_sb[:P, 6 + g:7 + g])
            dh1 = mlpp.tile([128, BT], F32, tag="mlpp", name="dh1t")
            nc.tensor.matmul(dh1[:P],
                             pa_sb[:P, OFF_W2TBD + 128 * g:OFF_W2TBD + 128 * g + P],
                             du2[:P], start=True, stop=True)
            du1 = du1p.tile([128, BT], sdt, tag="du1", name="du1t")
            nc.vector.tensor_mul(du1[:P], dh1[:P], sp1s[g][:P])
            du1_sb.append(du1)

        # ---- cross couplings + MLP d_z, b-major per 128-sample chunk
        for c in range(NCHUNK):
            bs = slice(128 * c, 128 * c + 128)
            op = [outp.tile([128, 512], F32, tag="outp", name="outpt")
                  for _ in range(4)]
            o8 = outp.tile([128, 512], F32, tag="outp", name="outpt")
            # blocks 0..7: dz_k = z_fut @ W_r (r = 8-k), + MLP term
            for k in range(P_MAX):
                pt = op[k // 2][:, 256 * (k % 2):256 * (k % 2) + 256]
                for ih in range(2):
                    zslice = zt_sb[t][:, BT * (2 * P_MAX + ih) + 128 * c:
                                      BT * (2 * P_MAX + ih) + 128 * c + 128]
                    wl = cp_sb[:, 256 * (2 * (7 - k) + ih):256 * (2 * (7 - k) + ih) + 256]
                    nc.tensor.matmul(pt, zslice, wl, start=(ih == 0), stop=False)
                g, j = k // 4, k % 4
                nc.tensor.matmul(
                    pt, du1_sb[g][32 * j:32 * j + 32, bs],
                    pa_sb[32 * j:32 * j + 32, OFF_W1T + 256 * g:OFF_W1T + 256 * g + 256],
                    start=False, stop=True, tile_position=(32 * j, 0))
            # block 8: dz_8 = sum_r lags_r @ W_r^T, + MLP term
            for r in range(1, P_MAX + 1):
                for jh in range(2):
                    zslice = zt_sb[t][:, BT * (2 * (P_MAX - r) + jh) + 128 * c:
                                      BT * (2 * (P_MAX - r) + jh) + 128 * c + 128]
                    wf = cp_sb[:, OFF_WFUT + 256 * (2 * (r - 1) + jh):
                               OFF_WFUT + 256 * (2 * (r - 1) + jh) + 256]
                    nc.tensor.matmul(o8[:, :NN], zslice, wf,
                                     start=(r == 1 and jh == 0), stop=False)
            nc.tensor.matmul(o8[:, :NN], du1_sb[2][0:32, bs],
                             pa_sb[0:32, OFF_W1T + 512:OFF_W1T + 512 + 256],
                             start=False, stop=True, tile_position=(0, 0))

            # ---- PSUM -> SBUF assembly, then one DMA for these 128 rows
            ot = outs.tile([128, D], F32, tag="outs", name="outst")
            for m in range(4):
                dst = ot[:, 512 * m:512 * (m + 1)]
                if m < 3:
                    nc.vector.tensor_copy(dst, op[m][:])
                else:
                    nc.scalar.activation(dst, op[m][:], AF.Copy)
            nc.scalar.activation(ot[:, 2048:2304], o8[:, :NN], AF.Copy)
            r0 = t * BT + c * 128
            nc.sync.dma_start(out[r0:r0 + 128, :], ot[:])


# ------------------------------------------------------------- build + launch

_CACHED = {}


def _build():
    if MM_MODE in _CACHED:
        return _CACHED[MM_MODE]
    sdt = _DT[MM_MODE][0]
    nc = bacc.Bacc("TRN2", target_bir_lowering=False, debug=False,
                   num_devices=N_CORES)
    zt = nc.dram_tensor("zt", [NBT, 128, ZW], sdt, kind="ExternalInput").ap()
    params = nc.dram_tensor("params", [128, PW], sdt, kind="ExternalInput").ap()
    cparams = nc.dram_tensor("cparams", [128, CW], sdt, kind="ExternalInput").ap()
    biases = nc.dram_tensor("biases", [128, 9], F32, kind="ExternalInput").ap()
    out = nc.dram_tensor("out", [B_CORE, D], F32, kind="ExternalOutput").ap()

    from contextlib import ExitStack
    with tile.TileContext(nc) as tc:
        with ExitStack() as ctx:
            _body(tc, out, zt, params, cparams, biases, ctx)
    nc.compile()
    _CACHED[MM_MODE] = nc
    return nc


def _prep_params(gW1, gb1, gW2, gb2, gW3, gb3, W):
    mdt = _DT[MM_MODE][1]
    params = np.zeros((128, PW), np.float32)
    biases = np.zeros((128, 9), np.float32)
    for k in range(NB):
        g, j = k // 4, k % 4
        rs = slice(32 * j, 32 * j + 32)
        for hf in range(2):
            # u1 lhsT piece (2k+hf): cols 32j..32j+32 hold W1_k[half]
            params[:, 128 * (2 * k + hf) + 32 * j:128 * (2 * k + hf) + 32 * j + 32] = \
                gW1[k, 128 * hf:128 * (hf + 1), :]
        params[rs, OFF_W1T + 256 * g:OFF_W1T + 256 * (g + 1)] = gW1[k].T
        params[rs, OFF_W2BD + 128 * g + 32 * j:OFF_W2BD + 128 * g + 32 * j + 32] = gW2[k]
        params[rs, OFF_W2TBD + 128 * g + 32 * j:OFF_W2TBD + 128 * g + 32 * j + 32] = gW2[k].T
        biases[rs, g] = gb1[k]
        biases[rs, 3 + g] = gb2[k]
        biases[rs, 6 + g] = -gW3[k]              # minus sign enters here
    cparams = np.zeros((128, CW), np.float32)
    for r in range(1, P_MAX + 1):
        for hf in range(2):
            i = 2 * (r - 1) + hf
            cparams[:, 256 * i:256 * (i + 1)] = -W[r - 1][128 * hf:128 * (hf + 1), :]
            cparams[:, OFF_WFUT + 256 * i:OFF_WFUT + 256 * (i + 1)] = \
                -W[r - 1].T[128 * hf:128 * (hf + 1), :]
    return {"params": params.astype(mdt), "cparams": cparams.astype(mdt),
            "biases": biases}


def run(inputs, trace=False):
    nc = _build()
    mdt = _DT[MM_MODE][1]
    params = _prep_params(
        np.asarray(inputs["gW1"]), np.asarray(inputs["gb1"]),
        np.asarray(inputs["gW2"]), np.asarray(inputs["gb2"]),
        np.asarray(inputs["gW3"]), np.asarray(inputs["gb3"]),
        np.asarray(inputs["W"]))
    z = np.asarray(inputs["z"])
    in_maps = []
    for ci in range(N_CORES):
        zc = z[ci * B_CORE:(ci + 1) * B_CORE]
        # zt[t, p, 512*c + b] = zc[512t + b, 128c + p]
        ztc = np.ascontiguousarray(
            zc.reshape(NBT, BT, 2 * NB, 128).transpose(0, 3, 2, 1)
        ).reshape(NBT, 128, ZW).astype(mdt)
        in_maps.append({"zt": ztc, **params})
    res = run_bass_kernel_spmd(nc, in_maps, core_ids=list(range(N_CORES)),
                               trace=trace)
    out = np.concatenate([r["out"] for r in res.results], axis=0)
    return out, res


def kernel(**inputs) -> np.ndarray:
    out, _ = run(inputs, trace=False)
    return out


# revision 10
# speedup vs baseline: 1.8869x; 1.1318x over previous
"""TRN2 Bass kernel for nn_MultiBlockStructuredScoreNet.

Computes s(z) = -grad_z U(z) where
  U(z) = sum_k MLP_k(z_k) + sum_r z_8^T W_r z_{8-r}
for z of shape (8192, 9*256), data-parallel over 8 NeuronCores.

Per core (1024 samples):
 - Host pre-transposes z to neuron-major (zT) so the PE contracts over the
   neuron dim with no on-chip transposes; host packs/transposes/sign-flips
   the small parameters so PSUM accumulation directly yields the score.
 - MM_MODE picks the PE dtype: fp16 (default: full PE rate, ~2.5e-4),
   bf16 (~2e-3), or f32r (fp32 storage, ~1.3e-4 but slower weight loads).
 - Inputs ship as 4 packed DRAM tensors -> ~10 big DMAs (per-DMA overhead
   on the HWDGE queues is ~0.6us, so DMA count matters more than bytes).
 - A short burst of throwaway matmuls right after the params DMA warms the
   PE clock gate (HAM) before the real work lands.
 - MLP: u1 via col-tiled [K=128,M=32] matmuls packing 4 blocks per PSUM
   tile (f32r falls back to stacked zero-padded weights: its matmuls must
   write PSUM partition 0); u2/dh1 via block-diagonal [128,128] weights
   with -gW3 pre-folded into the dh1 weights; SiLU / SiLU' from the ACT
   table with fused bias, batched by function to avoid table reloads.
 - Cross couplings + MLP d_z accumulate b-major into PSUM per 128-sample
   chunk; adjacent blocks are paired into single N=512 matmuls (d_lag via
   adjacent wlag packing, d_z via block-diagonal W1^T pairs) to halve the
   weight-load count.  PSUM->SBUF copies split across DVE/ACT; one 1.2MB
   DMA per chunk stores the final rows.
"""

import numpy as np
import ml_dtypes

import concourse.bass as bass
import concourse.tile as tile
from concourse import bacc, mybir
from concourse.bass_utils import run_bass_kernel_spmd

AF = mybir.ActivationFunctionType
F32 = mybir.dt.float32

N_CORES = 8
BATCH = 8192
B_CORE = BATCH // N_CORES     # 1024
BT = 512                      # batch tile (PSUM free-dim max for f32)
NBT = B_CORE // BT            # 2 batch tiles per core
NCHUNK = 4                    # 128-sample chunks per batch tile
NB = 9                        # blocks
P_MAX = 8
NN = 256                      # neurons per block
D = NB * NN                   # 2304
H = 32

ZW = 2 * NB * BT              # zt tile cols per batch tile: 18 chunks x 512
OFF_W1T = 2 * NB * 128        # 2304: paired block-diag W1^T region
OFF_W2BD = OFF_W1T + 4 * 512 + 256   # 4608
OFF_W2TBD = OFF_W2BD + 3 * 128       # 4992
PW = OFF_W2TBD + 3 * 128             # 5376
OFF_WFUT = 2 * P_MAX * NN     # 4096
CW = 2 * OFF_WFUT             # 8192

MM_MODE = "fp16"              # "fp16" | "bf16" | "f32r"
WARMUP_MMS = 12

_DT = {
    "fp16": (mybir.dt.float16, np.float16),
    "bf16": (mybir.dt.bfloat16, ml_dtypes.bfloat16),
    "f32r": (mybir.dt.float32r, np.float32),
}


def _body(tc, out, zt, params, cparams, biases, ctx):
    nc = tc.nc
    sdt = _DT[MM_MODE][0]

    const = ctx.enter_context(tc.tile_pool(name="const", bufs=1))
    ztp = ctx.enter_context(tc.tile_pool(name="ztp", bufs=2))
    mlpp = ctx.enter_context(tc.tile_pool(name="mlpp", bufs=3, space="PSUM"))
    actp = ctx.enter_context(tc.tile_pool(name="actp", bufs=8))
    du1p = ctx.enter_context(tc.tile_pool(name="du1p", bufs=6))
    outp = ctx.enter_context(tc.tile_pool(name="outp", bufs=5, space="PSUM"))
    outs = ctx.enter_context(tc.tile_pool(name="outs", bufs=3))

    # ---- inputs -> SBUF (few big DMAs; ordered so compute starts early)
    pa_sb = const.tile([128, PW], sdt, name="pa")
    nc.sync.dma_start(pa_sb[:], params[:])
    bias_sb = const.tile([128, 6], F32, name="biassb")
    nc.sync.dma_start(bias_sb[:], biases[:])
    zt_sb = [ztp.tile([128, ZW], sdt, tag="zt", name="ztsb") for _ in range(NBT)]
    SPLITS = [(0, 8 * BT), (8 * BT, 16 * BT), (16 * BT, ZW)]
    for a, b in SPLITS:
        nc.sync.dma_start(zt_sb[0][:, a:b], zt[0, :, a:b])
    cp_sb = const.tile([128, CW], sdt, name="cp")
    nc.sync.dma_start(cp_sb[:], cparams[:])
    for a, b in SPLITS:
        nc.sync.dma_start(zt_sb[1][:, a:b], zt[1, :, a:b])

    def ztsl(t, c, c0, w):    # [128, w] slice at offset c0 of zT chunk c
        return zt_sb[t][:, BT * c + c0:BT * c + c0 + w]

    # ---- HAM warm-up: throwaway matmuls on the params tile
    wu = mlpp.tile([128, BT], F32, tag="mlpp", name="wut")
    for i in range(WARMUP_MMS):
        nc.tensor.matmul(wu[:], pa_sb[:, 0:128], pa_sb[:, 0:BT],
                         start=True, stop=True)

    for t in range(NBT):
        # ---- per-block MLPs, batched by ACT function across the 3 groups
        u1s, h1s, sp1s = [], [], []
        for g in range(3):
            nblk = 4 if g < 2 else 1
            P = 32 * nblk
            u1 = mlpp.tile([128, BT], F32, tag="mlpp", name="u1t")
            for j in range(nblk):
                k = 4 * g + j
                for hf in range(2):
                    c = 2 * k + hf
                    if MM_MODE == "f32r":
                        nc.tensor.matmul(
                            u1[:P, :], pa_sb[:, 128 * c:128 * c + P],
                            ztsl(t, c, 0, BT),
                            start=(2 * j + hf == 0), stop=(2 * j + hf == 2 * nblk - 1))
                    else:
                        nc.tensor.matmul(
                            u1[32 * j:32 * j + 32, :],
                            pa_sb[:, 128 * c + 32 * j:128 * c + 32 * j + 32],
                            ztsl(t, c, 0, BT), start=(hf == 0), stop=(hf == 1),
                            tile_position=(0, 32 * j))
            u1s.append((u1, P))
        for g in range(3):
            u1, P = u1s[g]
            h1 = actp.tile([128, BT], sdt, tag="act", name="h1t")
            nc.scalar.activation(h1[:P], u1[:P], AF.Silu, bias=bias_sb[:P, g:g + 1])
            h1s.append(h1)
        for g in range(3):
            u1, P = u1s[g]
            sp1 = actp.tile([128, BT], F32, tag="act", name="sp1t")
            nc.scalar.activation(sp1[:P], u1[:P], AF.Derivative_silu,
                                 bias=bias_sb[:P, g:g + 1])
            sp1s.append(sp1)
        du1_sb = []
        for g in range(3):
            P = u1s[g][1]
            u2 = mlpp.tile([128, BT], F32, tag="mlpp", name="u2t")
            nc.tensor.matmul(u2[:P], pa_sb[:P, OFF_W2BD + 128 * g:OFF_W2BD + 128 * g + P],
                             h1s[g][:P], start=True, stop=True)
            sp2 = actp.tile([128, BT], sdt, tag="act", name="sp2t")
            nc.scalar.activation(sp2[:P], u2[:P], AF.Derivative_silu,
                                 bias=bias_sb[:P, 3 + g:4 + g])
            dh1 = mlpp.tile([128, BT], F32, tag="mlpp", name="dh1t")
            # -gW3 is folded into these weights: dh1 here is -d(e)/d(h1)
            nc.tensor.matmul(dh1[:P],
                             pa_sb[:P, OFF_W2TBD + 128 * g:OFF_W2TBD + 128 * g + P],
                             sp2[:P], start=True, stop=True)
            du1 = du1p.tile([128, BT], sdt, tag="du1", name="du1t")
            nc.vector.tensor_mul(du1[:P], dh1[:P], sp1s[g][:P])
            du1_sb.append(du1)

        # ---- cross couplings + MLP d_z, b-major per 128-sample chunk
        for c in range(NCHUNK):
            bs = slice(128 * c, 128 * c + 128)
            op = [outp.tile([128, 512], F32, tag="outp", name="outpt")
                  for _ in range(4)]
            o8 = outp.tile([128, 512], F32, tag="outp", name="outpt")
            # paired d_lag: blocks (2p, 2p+1) <- z_fut @ [-W_{8-2p} | -W_{7-2p}]
            for p in range(4):
                for ih in range(2):
                    nc.tensor.matmul(
                        op[p][:], ztsl(t, 2 * P_MAX + ih, 128 * c, 128),
                        cp_sb[:, 2048 * ih + 512 * p:2048 * ih + 512 * p + 512],
                        start=(ih == 0), stop=False)
            # d_future accumulation: sum_r lags_r @ -W_r^T
            for r in range(1, P_MAX + 1):
                for jh in range(2):
                    i = 2 * (r - 1) + jh
                    nc.tensor.matmul(
                        o8[:, :NN], ztsl(t, 2 * (P_MAX - r) + jh, 128 * c, 128),
                        cp_sb[:, OFF_WFUT + 256 * i:OFF_WFUT + 256 * i + 256],
                        start=(i == 0), stop=False)
            # paired MLP d_z: du1 pairs @ block-diag W1^T
            for m in range(4):
                base = 64 * (m % 2)
                nc.tensor.matmul(
                    op[m][:], du1_sb[m // 2][base:base + 64, bs],
                    pa_sb[base:base + 64, OFF_W1T + 512 * m:OFF_W1T + 512 * m + 512],
                    start=False, stop=True, tile_position=(base, 0))
            nc.tensor.matmul(o8[:, :NN], du1_sb[2][0:32, bs],
                             pa_sb[0:32, OFF_W1T + 2048:OFF_W1T + 2048 + 256],
                             start=False, stop=True, tile_position=(0, 0))

            # ---- PSUM -> SBUF assembly, then one DMA for these 128 rows
            ot = outs.tile([128, D], F32, tag="outs", name="outst")
            for m in range(3):
                nc.vector.tensor_copy(ot[:, 512 * m:512 * (m + 1)], op[m][:])
            nc.scalar.activation(ot[:, 1536:2048], op[3][:], AF.Copy)
            nc.scalar.activation(ot[:, 2048:2304], o8[:, :NN], AF.Copy)
            r0 = t * BT + c * 128
            nc.sync.dma_start(out[r0:r0 + 128, :], ot[:])


# ------------------------------------------------------------- build + launch

_CACHED = {}


def _build():
    if MM_MODE in _CACHED:
        return _CACHED[MM_MODE]
    sdt = _DT[MM_MODE][0]
    nc = bacc.Bacc("TRN2", target_bir_lowering=False, debug=False,
                   num_devices=N_CORES)
    zt = nc.dram_tensor("zt", [NBT, 128, ZW], sdt, kind="ExternalInput").ap()
    params = nc.dram_tensor("params", [128, PW], sdt, kind="ExternalInput").ap()
    cparams = nc.dram_tensor("cparams", [128, CW], sdt, kind="ExternalInput").ap()
    biases = nc.dram_tensor("biases", [128, 6], F32, kind="ExternalInput").ap()
    out = nc.dram_tensor("out", [B_CORE, D], F32, kind="ExternalOutput").ap()

    from contextlib import ExitStack
    with tile.TileContext(nc) as tc:
        with ExitStack() as ctx:
            _body(tc, out, zt, params, cparams, biases, ctx)
    nc.compile()
    _CACHED[MM_MODE] = nc
    return nc


def _prep_params(gW1, gb1, gW2, gb2, gW3, gb3, W):
    mdt = _DT[MM_MODE][1]
    params = np.zeros((128, PW), np.float32)
    biases = np.zeros((128, 6), np.float32)
    for k in range(NB):
        g, j = k // 4, k % 4
        rs = slice(32 * j, 32 * j + 32)
        for hf in range(2):
            # u1 lhsT piece (2k+hf): cols 32j..32j+32 hold W1_k[half]
            params[:, 128 * (2 * k + hf) + 32 * j:128 * (2 * k + hf) + 32 * j + 32] = \
                gW1[k, 128 * hf:128 * (hf + 1), :]
        params[rs, OFF_W2BD + 128 * g + 32 * j:OFF_W2BD + 128 * g + 32 * j + 32] = gW2[k]
        # dh1 weights with -gW3 folded in:
        # lhsT[32j+g', 32j+h] = -gW3[k][g'] * gW2[k][h, g']
        params[rs, OFF_W2TBD + 128 * g + 32 * j:OFF_W2TBD + 128 * g + 32 * j + 32] = \
            -gW3[k][:, None] * gW2[k].T
        biases[rs, g] = gb1[k]
        biases[rs, 3 + g] = gb2[k]
    # paired block-diagonal W1^T for the d_z matmuls (not negated: du1
    # already carries the sign flip from the folded -gW3)
    for m in range(4):
        base = 64 * (m % 2)
        for s in range(2):
            k = 2 * m + s
            params[base + 32 * s:base + 32 * s + 32,
                   OFF_W1T + 512 * m + 256 * s:OFF_W1T + 512 * m + 256 * (s + 1)] = \
                gW1[k].T
    params[0:32, OFF_W1T + 2048:OFF_W1T + 2048 + 256] = gW1[8].T

    cparams = np.zeros((128, CW), np.float32)
    for ih in range(2):
        for k in range(P_MAX):            # block k pairs with lag r = 8-k
            cparams[:, 2048 * ih + 256 * k:2048 * ih + 256 * (k + 1)] = \
                -W[7 - k][128 * ih:128 * (ih + 1), :]
    for r in range(1, P_MAX + 1):
        for jh in range(2):
            i = 2 * (r - 1) + jh
            cparams[:, OFF_WFUT + 256 * i:OFF_WFUT + 256 * (i + 1)] = \
                -W[r - 1].T[128 * jh:128 * (jh + 1), :]
    return {"params": params.astype(mdt), "cparams": cparams.astype(mdt),
            "biases": biases}


def run(inputs, trace=False):
    nc = _build()
    mdt = _DT[MM_MODE][1]
    params = _prep_params(
        np.asarray(inputs["gW1"]), np.asarray(inputs["gb1"]),
        np.asarray(inputs["gW2"]), np.asarray(inputs["gb2"]),
        np.asarray(inputs["gW3"]), np.asarray(inputs["gb3"]),
        np.asarray(inputs["W"]))
    z = np.asarray(inputs["z"])
    in_maps = []
    for ci in range(N_CORES):
        zc = z[ci * B_CORE:(ci + 1) * B_CORE]
        # zt[t, p, 512*c + b] = zc[512t + b, 128c + p]
        ztc = np.ascontiguousarray(
            zc.reshape(NBT, BT, 2 * NB, 128).transpose(0, 3, 2, 1)
        ).reshape(NBT, 128, ZW).astype(mdt)
        in_maps.append({"zt": ztc, **params})
    res = run_bass_kernel_spmd(nc, in_maps, core_ids=list(range(N_CORES)),
                               trace=trace)
    out = np.concatenate([r["out"] for r in res.results], axis=0)
    return out, res


def kernel(**inputs) -> np.ndarray:
    out, _ = run(inputs, trace=False)
    return out
